# revision 7
# baseline (speedup 1.0000x reference)
"""Causal self-attention block (B=4, T=2048, C=2048, H=16, D=128) on 8 trn2 cores.

Sharding: tensor-parallel over head groups (2 groups of 8 heads) x
data-parallel over batch (4). Core (g, b) computes, for batch b and heads
[8g, 8g+8): qkv projection, causal attention, and the partial output
projection contribution attn_out[:, heads_g] @ Wproj[rows_g]. The host sums
the two partial yT per batch, adds bproj, and transposes back.

All matmuls run as float32r (fp32 rounded to 12-bit mantissa; exact on
pre-rounded inputs at full PE speed). Everything on-chip is kept in
transposed layouts so no fp32 DMA-transposes are needed:
  phase 1: qkvT[col, t] = W^T x^T  (24 col-tiles of 128, t in chunks of 512)
  phase 2: per head: S^T = K^T^T Q^T blocks -> exp -> causal mask; column
           sums via ones-matmul; out^T = V^T P^T; normalize by 1/colsum.
           V (natural layout, AV lhsT) comes from 128x128 PE transposes of V^T.
  phase 3: yT = Wproj_g^T attn_outT  (accumulate over the 8 head chunks)
"""

import sys

sys.path.insert(0, "/opt/trn_rl_repo")

import numpy as np

import concourse.bass as bass
import concourse.mybir as mybir
import concourse.tile as tile
from concourse import bacc
from concourse.bass_utils import run_bass_kernel_spmd
from concourse.masks import make_identity

F32 = mybir.dt.float32
F32R = mybir.dt.float32r
AF = mybir.ActivationFunctionType

B, T, C = 4, 2048, 2048
H, D = 16, 128
G = 2  # head-group shards
HPC = H // G  # heads per core = 8
CT = C // 128  # contraction chunks = 16
NT = T // 512  # t chunks of 512 = 4
NJ = 3 * HPC  # qkv col tiles per core = 24
SCALE = 1.0 / float(np.sqrt(D))


def round_fp32r(x: np.ndarray) -> np.ndarray:
    """Round fp32 to fp32r (low 12 mantissa bits, round-to-nearest-even)."""
    u = np.ascontiguousarray(x, dtype=np.float32).view(np.uint32).astype(np.uint64)
    r = (u + 0x7FF + ((u >> 12) & 1)) & ~np.uint64(0xFFF)
    return r.astype(np.uint32).view(np.float32).reshape(x.shape)


def build_nc():
    nc = bacc.Bacc("TRN2", target_bir_lowering=False)
    xT = nc.dram_tensor("xT", [128, CT, T], F32R, kind="ExternalInput")
    wqkv = nc.dram_tensor("wqkv", [128, NJ, CT, 128], F32R, kind="ExternalInput")
    wproj = nc.dram_tensor("wproj", [128, CT, HPC, 128], F32R, kind="ExternalInput")
    bqkv = nc.dram_tensor("bqkv", [128, NJ], F32, kind="ExternalInput")
    masks = nc.dram_tensor("masks", [128, 4, 512], F32R, kind="ExternalInput")
    yT = nc.dram_tensor("yT", [C, T], F32, kind="ExternalOutput")
    yT_r = yT.rearrange("(i p) t -> p i t", p=128)

    with tile.TileContext(nc) as tc:
        with (
            tc.tile_pool(name="const", bufs=1) as cst,
            tc.tile_pool(name="dram", bufs=1, space="DRAM") as dram,
        ):
            masks_sb = cst.tile([128, 4, 512], F32R)
            nc.sync.dma_start(masks_sb, masks.ap())
            bias_sb = cst.tile([128, NJ], F32)
            nc.sync.dma_start(bias_sb, bqkv.ap())
            ident = cst.tile([128, 128], F32)
            make_identity(nc, ident)
            ones_f = cst.tile([128, 2], F32)
            nc.vector.memset(ones_f, 1.0)
            ones = cst.tile([128, 2], F32R)
            nc.vector.tensor_copy(ones, ones_f)

            qkvT = [
                dram.tile([128, T], F32R, name=f"qkvT{j}", tag=f"qkvT{j}")
                for j in range(NJ)
            ]
            outT = [
                dram.tile([128, T], F32R, name=f"outT{h}", tag=f"outT{h}")
                for h in range(HPC)
            ]


            # ---------------- phase 1: qkvT[col, t] = W^T x^T (+bias) -------
            with (
                tc.tile_pool(name="p1x", bufs=1) as p1x,
                tc.tile_pool(name="p1w", bufs=3) as p1w,
                tc.tile_pool(name="p1s", bufs=4) as p1s,
                tc.tile_pool(name="ps1", bufs=4, space="PSUM") as ps1,
            ):
                xs = p1x.tile([128, CT, T], F32R)
                nc.sync.dma_start(xs, xT.ap())
                for j in range(NJ):
                    w_sb = p1w.tile([128, CT, 128], F32R, tag="w")
                    nc.sync.dma_start(w_sb, wqkv[:, j])
                    for c in range(NT):
                        ps = ps1.tile([128, 512], F32, tag="ps")
                        for cc in range(CT):
                            nc.tensor.matmul(
                                ps,
                                lhsT=w_sb[:, cc, :],
                                rhs=xs[:, cc, 512 * c : 512 * (c + 1)],
                                start=(cc == 0),
                                stop=(cc == CT - 1),
                            )
                        st = p1s.tile([128, 512], F32R, tag="st")
                        nc.vector.tensor_scalar_add(st, ps, bias_sb[:, j : j + 1])
                        nc.sync.dma_start(qkvT[j][:, 512 * c : 512 * (c + 1)], st)

            # ---------------- phase 2: per-head causal attention ------------
            with (
                tc.tile_pool(name="p2qk", bufs=2) as p2qk,
                tc.tile_pool(name="p2v", bufs=2) as p2v,
                tc.tile_pool(name="p2p", bufs=2) as p2p,
                tc.tile_pool(name="p2sc", bufs=4) as p2sc,
                tc.tile_pool(name="p2o", bufs=3) as p2o,
                tc.tile_pool(name="ps2s", bufs=3, space="PSUM") as ps2s,
                tc.tile_pool(name="ps2t", bufs=2, space="PSUM") as ps2t,
                tc.tile_pool(name="ps2m", bufs=1, space="PSUM") as ps2m,
                tc.tile_pool(name="ps2o", bufs=2, space="PSUM") as ps2o,
                tc.tile_pool(name="dram_rb", bufs=4, space="DRAM") as dram_rb,
            ):
                for h in range(HPC):
                    q_sb = p2qk.tile([128, T], F32R, tag="q")
                    nc.sync.dma_start(q_sb, qkvT[h][:])
                    k_sb = p2qk.tile([128, T], F32R, tag="k")
                    nc.sync.dma_start(k_sb, qkvT[HPC + h][:])
                    vt_sb = p2qk.tile([128, T], F32R, tag="vt")
                    nc.sync.dma_start(vt_sb, qkvT[2 * HPC + h][:])

                    # V natural layout via PE transposes of V^T 128x128 blocks
                    v_sb = p2v.tile([128, T // 128, 128], F32R, tag="v")
                    for j in range(T // 128):
                        ps_v = ps2t.tile([128, 128], F32, tag="pst")
                        nc.tensor.transpose(
                            ps_v, vt_sb[:, 128 * j : 128 * (j + 1)].bitcast(F32), ident
                        )
                        nc.vector.tensor_copy(v_sb[:, j, :], ps_v)

                    for c in range(NT):
                        nblk = 4 * c + 4
                        pT = p2p.tile([128, T // 128, 512], F32R, tag="pT")
                        for j in range(nblk):
                            ps_s = ps2s.tile([128, 512], F32, tag="s")
                            nc.tensor.matmul(
                                ps_s,
                                lhsT=k_sb[:, 128 * j : 128 * (j + 1)],
                                rhs=q_sb[:, 512 * c : 512 * (c + 1)],
                                start=True,
                                stop=True,
                            )
                            nc.scalar.activation(
                                pT[:, j, :], ps_s, AF.Exp, scale=SCALE
                            )
                            if j >= 4 * c:
                                nc.vector.tensor_mul(
                                    pT[:, j, :], pT[:, j, :], masks_sb[:, j - 4 * c, :]
                                )
                        ps_sum = ps2m.tile([2, 512], F32, tag="sum")
                        for j in range(nblk):
                            nc.tensor.matmul(
                                ps_sum,
                                lhsT=ones,
                                rhs=pT[:, j, :],
                                start=(j == 0),
                                stop=(j == nblk - 1),
                            )
                        rs = p2sc.tile([1, 512], F32, tag="rs")
                        nc.vector.reciprocal(rs, ps_sum[0:1, :])
                        rbx = dram_rb.tile([1, 512], F32, tag="rbx")
                        nc.sync.dma_start(rbx, rs)
                        rb = p2sc.tile([128, 512], F32, tag="rb")
                        nc.gpsimd.dma_start(rb, rbx[0].partition_broadcast(128))
                        ps_o = ps2o.tile([128, 512], F32, tag="o")
                        for j in range(nblk):
                            nc.tensor.matmul(
                                ps_o,
                                lhsT=v_sb[:, j, :],
                                rhs=pT[:, j, :],
                                start=(j == 0),
                                stop=(j == nblk - 1),
                            )
                        ot = p2o.tile([128, 512], F32R, tag="ot")
                        nc.vector.tensor_mul(ot, ps_o, rb)
                        nc.sync.dma_start(outT[h][:, 512 * c : 512 * (c + 1)], ot)

            # ---------------- phase 3: yT = Wproj_g^T attn_outT -------------
            with (
                tc.tile_pool(name="p3o", bufs=1) as p3o,
                tc.tile_pool(name="p3w", bufs=3) as p3w,
                tc.tile_pool(name="p3y", bufs=4) as p3y,
                tc.tile_pool(name="ps3", bufs=4, space="PSUM") as ps3,
            ):
                o_sb = []
                for h in range(HPC):
                    t = p3o.tile([128, T], F32R, name=f"osb{h}", tag=f"osb{h}")
                    nc.sync.dma_start(t, outT[h][:])
                    o_sb.append(t)
                for i in range(CT):
                    wp = p3w.tile([128, HPC, 128], F32R, tag="wp")
                    nc.sync.dma_start(wp, wproj[:, i])
                    for c in range(NT):
                        ps_y = ps3.tile([128, 512], F32, tag="y")
                        for hh in range(HPC):
                            nc.tensor.matmul(
                                ps_y,
                                lhsT=wp[:, hh, :],
                                rhs=o_sb[hh][:, 512 * c : 512 * (c + 1)],
                                start=(hh == 0),
                                stop=(hh == HPC - 1),
                            )
                        ys = p3y.tile([128, 512], F32, tag="ys")
                        nc.vector.tensor_copy(ys, ps_y)
                        nc.sync.dma_start(yT_r[:, i, 512 * c : 512 * (c + 1)], ys)
    nc.compile()
    return nc


_NC_CACHE = None


def _get_nc():
    global _NC_CACHE
    if _NC_CACHE is None:
        _NC_CACHE = build_nc()
    return _NC_CACHE


def _prep_inputs(x, Wqkv, bqkv, Wproj):
    """Host-side shard + pre-tile + fp32r-round. Returns list of 8 in_maps,
    core index = g * B + b."""
    x = round_fp32r(np.asarray(x))
    Wqkv = round_fp32r(np.asarray(Wqkv))
    Wproj = round_fp32r(np.asarray(Wproj))
    bqkv = np.asarray(bqkv, dtype=np.float32)

    # causal mask variants for the diagonal 512-chunks
    p = np.arange(128)[:, None]
    f = np.arange(512)[None, :]
    masks = np.stack(
        [(f >= 128 * v + p).astype(np.float32) for v in range(4)], axis=1
    )  # [128, 4, 512]
    masks = np.ascontiguousarray(masks)

    # xT tiles per batch: [128, CT, T] with [p, o, t] = x[b, t, o*128+p]
    xT_b = []
    for b in range(B):
        xt = np.ascontiguousarray(x[b].T)  # [C, T]
        xT_b.append(np.ascontiguousarray(xt.reshape(CT, 128, T).transpose(1, 0, 2)))

    in_maps = [None] * (G * B)
    for g in range(G):
        cols = np.concatenate(
            [
                np.arange(g * 1024, (g + 1) * 1024),
                np.arange(C + g * 1024, C + (g + 1) * 1024),
                np.arange(2 * C + g * 1024, 2 * C + (g + 1) * 1024),
            ]
        )
        wg = Wqkv[:, cols]  # [C, 3072] = [(o p), (j m)]
        # -> [128 p, 24 j, 16 o, 128 m]
        wg_t = np.ascontiguousarray(
            wg.reshape(CT, 128, NJ, 128).transpose(1, 2, 0, 3)
        )
        bg = bqkv[cols]  # [3072]
        bg_t = np.ascontiguousarray(bg.reshape(NJ, 128).T)  # [128, 24]
        wp = Wproj[g * 1024 : (g + 1) * 1024, :]  # [1024, C] = [(h p), (i m)]
        # -> [128 p, 16 i, 8 h, 128 m]
        wp_t = np.ascontiguousarray(
            wp.reshape(HPC, 128, CT, 128).transpose(1, 2, 0, 3)
        )
        for b in range(B):
            in_maps[g * B + b] = dict(
                xT=xT_b[b], wqkv=wg_t, wproj=wp_t, bqkv=bg_t, masks=masks
            )
    return in_maps


def kernel(x, Wqkv, bqkv, Wproj, bproj):
    x = np.asarray(x)
    nc = _get_nc()
    in_maps = _prep_inputs(x, Wqkv, bqkv, Wproj)
    res = run_bass_kernel_spmd(nc, in_maps, core_ids=list(range(G * B)))
    y = np.empty((B, T, C), dtype=np.float32)
    bp = np.asarray(bproj, dtype=np.float32)
    for b in range(B):
        acc = res.results[b]["yT"].astype(np.float32).copy()
        for g in range(1, G):
            acc += res.results[g * B + b]["yT"]
        y[b] = acc.T + bp[None, :]
    return y
